# revision 1
# baseline (speedup 1.0000x reference)
"""MemoryBank kernel v3 for 8x TRN2 NeuronCores (SPMD, batch-parallel).

Score matmul in split precision (fp16 main + one fp8 DoubleRow correction
pass), since f32r loses too much mantissa for the sharp softmax and plain
fp32 runs quarter-rate on the PE:

    x  = xh (fp16, DMA'd) + xl (e4m3 * 2^-10, DMA'd)
    G  = Gh (fp16)        + Gl (e4m3 * 2^-10, on-chip constant)
    S  = xh@Gh  (fp16 full-rate)
       + 2^-10 * [ (xl*2^10)@e4m3(Gh) + e4m3(xh)@(Gl*2^10) ]   (fp8 DoubleRow)

e4m3(xh) is derived on-chip by gpsimd/ACT copies (idle engines), so x DMA
is 24 MiB/core instead of 32. Gate folding as v2:

    en = exp(-gate_logit); g = 1/(1+en); Ec' = -E*r*en
    PSUM = xh + Ec'@WvN   (identity-pass + retrieval on PE)
    out  = g * PSUM       (single DVE op per element, fp16 out -> 16 MiB)

DMA floor: (16+8+16) MiB / 358 GB/s ~= 117 us/core.
"""

from contextlib import ExitStack

import numpy as np

import concourse.bass as bass
import concourse.tile as tile
from concourse import bacc
from concourse import mybir
from concourse.bass import ts
from concourse.bass_utils import run_bass_kernel_spmd
from concourse.masks import make_identity

F32 = mybir.dt.float32
F32R = mybir.dt.float32r
F16 = mybir.dt.float16
F8 = mybir.dt.float8e4
AX_X = mybir.AxisListType
ALU = mybir.AluOpType
ACTF = mybir.ActivationFunctionType
DR = mybir.MatmulPerfMode.DoubleRow

B = 8
L = 4096
DIM = 2048
NSLOT = 64
NCH = DIM // 128  # 16 dim chunks
TOK = 512  # tokens per tile
NT = L // TOK  # 8 tiles per core
NQ = TOK // 128  # 4 token quarters per tile
CSCALE = 1024.0  # 2^10 scale of the fp8 correction pass
MPAD = 80  # fp8 DoubleRow weight APs need step%16==0, so pad 65 -> 80 cols


def _build(gate_b: float, score_mode: str = "split8", cvt_pool: int = 16) -> bass.Bass:
    nc = bacc.Bacc("TRN2", target_bir_lowering=False, debug=False)
    split8 = score_mode == "split8"
    XD = F16 if split8 else F32

    xT = nc.dram_tensor("xT", [DIM, L], XD, kind="ExternalInput").ap()
    GT = nc.dram_tensor("GT", [DIM, NSLOT + 1], XD, kind="ExternalInput").ap()
    WvN = nc.dram_tensor("WvN", [NSLOT, DIM], F16, kind="ExternalInput").ap()
    gv = nc.dram_tensor("gv", [1, NSLOT], F32, kind="ExternalInput").ap()
    if split8:
        xl = nc.dram_tensor("xl", [DIM, L], F8, kind="ExternalInput").ap()
        xh8 = nc.dram_tensor("xh8", [DIM, L], F8, kind="ExternalInput").ap()
        GC = nc.dram_tensor("GC", [2 * DIM, MPAD], F8, kind="ExternalInput").ap()
        xl_v = xl.rearrange("(c p) t -> p c t", p=128)  # [128, 16, L]
        xh8_v = xh8.rearrange("(c p) t -> p c t", p=128)  # [128, 16, L]
        GC_v = GC.rearrange("(s p) m -> p s m", p=128)  # [128, 32, 80]
    outT = nc.dram_tensor("outT", [DIM, L], F16, kind="ExternalOutput").ap()

    # dim d = c*128 + p  (chunk-major split; consistent everywhere)
    xT_v = xT.rearrange("(c p) t -> p c t", p=128)  # [128, 16, L]
    GT_v = GT.rearrange("(c p) m -> p c m", p=128)  # [128, 16, 65]
    outT_v = outT.rearrange("(c p) t -> p c t", p=128)
    WvN_v = WvN.rearrange("n (c q) -> n c q", q=128)  # [64, 16, 128]

    with tile.TileContext(nc) as tc, ExitStack() as ctx:
        consts = ctx.enter_context(tc.tile_pool(name="consts", bufs=1))
        xpool = ctx.enter_context(tc.tile_pool(name="xpool", bufs=4))
        qpool = ctx.enter_context(tc.tile_pool(name="qpool", bufs=3))
        opool = ctx.enter_context(tc.tile_pool(name="opool", bufs=2))
        work = ctx.enter_context(tc.tile_pool(name="work", bufs=3))
        small = ctx.enter_context(tc.tile_pool(name="small", bufs=3))
        psA = ctx.enter_context(tc.tile_pool(name="psA", bufs=1, space="PSUM"))
        psA2 = ctx.enter_context(tc.tile_pool(name="psA2", bufs=1, space="PSUM"))
        psT = ctx.enter_context(tc.tile_pool(name="psT", bufs=1, space="PSUM"))
        psE = ctx.enter_context(tc.tile_pool(name="psE", bufs=1, space="PSUM"))
        psR = ctx.enter_context(tc.tile_pool(name="psR", bufs=3, space="PSUM"))
        psG = ctx.enter_context(tc.tile_pool(name="psG", bufs=1, space="PSUM"))

        ident = consts.tile([128, 128], F32)
        make_identity(nc, ident)
        ident16 = consts.tile([128, 128], F16)
        nc.scalar.copy(ident16, ident)
        ident_i = ident16 if split8 else ident.bitcast(F32R)
        GT_sb = consts.tile([128, NCH, NSLOT + 1], XD)
        nc.sync.dma_start(out=GT_sb, in_=GT_v)
        if split8:
            GC_sb = consts.tile([128, 2 * NCH, MPAD], F8)
            nc.sync.dma_start(out=GC_sb, in_=GC_v)
        Wv_sb = consts.tile([NSLOT, NCH, 128], F16)
        nc.sync.dma_start(out=Wv_sb, in_=WvN_v)
        gv_rep = consts.tile([128, NSLOT], F32)
        nc.sync.dma_start(out=gv_rep, in_=gv.to_broadcast((128, NSLOT)))
        ones16 = consts.tile([NSLOT + 1, 128], F16)
        nc.vector.memset(ones16, 1.0)

        def phase_A_dma(t, split=False):
            """prefetch x tile (3 tiles ahead of use)."""
            st = {}
            x_sb = xpool.tile([128, NCH, TOK], XD, tag="x_sb")
            if split:
                for h in range(4):
                    nc.sync.dma_start(
                        out=x_sb[:, 4 * h : 4 * h + 4, :],
                        in_=xT_v[:, 4 * h : 4 * h + 4, ts(t, TOK)],
                    )
            else:
                nc.sync.dma_start(out=x_sb, in_=xT_v[:, :, ts(t, TOK)])
            st["x_sb"] = x_sb
            if split8:
                xq8 = qpool.tile([128, 2 * NCH, TOK], F8, tag="xq8")
                nc.sync.dma_start(out=xq8[:, 0:NCH, :], in_=xl_v[:, :, ts(t, TOK)])
                nc.sync.dma_start(
                    out=xq8[:, NCH : 2 * NCH, :], in_=xh8_v[:, :, ts(t, TOK)]
                )
                st["xq8"] = xq8
            return st

        def S_corr_mms(st, S2_ps):
            """fp8 DoubleRow pass: (xl*2^10)@e4m3(Gh) + e4m3(xh)@(Gl*2^10)."""
            xq8 = st["xq8"]
            for i in range(NCH):
                nc.tensor.matmul(
                    S2_ps[0:MPAD, :],
                    GC_sb[:, 2 * i : 2 * i + 2, :],
                    xq8[:, 2 * i : 2 * i + 2, :],
                    start=(i == 0),
                    stop=(i == NCH - 1),
                    perf_mode=DR,
                    skip_group_check=True,
                )

        def S_merge(st, S_ps, S2_ps):
            """S_sb = main + corr * 2^-10, staged for the transposes."""
            if split8:
                S_c = work.tile([NSLOT + 1, TOK], F32, tag="S_c")
                nc.scalar.activation(
                    S_c, S2_ps[0 : NSLOT + 1, :], func=ACTF.Copy, scale=1.0 / CSCALE
                )
                S_sb = work.tile([NSLOT + 1, TOK], F32, tag="S_sb")
                nc.vector.tensor_add(S_sb, S_ps[0 : NSLOT + 1, :], S_c)
            else:
                S_sb = work.tile([NSLOT + 1, TOK], F32, tag="S_sb")
                nc.scalar.copy(S_sb, S_ps[0 : NSLOT + 1, :])
            Stok = psT.tile([128, NQ, NSLOT + 1], F32, tag="T")
            for q in range(NQ):
                nc.tensor.transpose(
                    Stok[:, q, :],
                    S_sb[:, ts(q, 128)],
                    ident[0 : NSLOT + 1, 0 : NSLOT + 1],
                )
            st["Stok"] = Stok

        def phase_A_mm(t, st):
            """S matmuls (main fp16 + fp8 corr) -> merge -> transposes."""
            x_sb = st["x_sb"]
            S_ps = psA.tile([128, TOK], F32, tag="A")
            for c in range(NCH):
                nc.tensor.matmul(
                    S_ps[0 : NSLOT + 1, :],
                    GT_sb[:, c, :],
                    x_sb[:, c, :],
                    start=(c == 0),
                    stop=(c == NCH - 1),
                )
            if split8:
                S2_ps = psA2.tile([128, TOK], F32, tag="A2")
                S_corr_mms(st, S2_ps)
            else:
                S2_ps = None
            S_merge(st, S_ps, S2_ps)

        def phase_B(t, st):
            """Batched softmax/gate stats; Ec' = -E*r*en in fp16, g in row 64."""
            Stok = st["Stok"]
            Etok = [
                small.tile([128, NSLOT], F32, tag=f"Etok{q}", name=f"Etok{q}")
                for q in range(NQ)
            ]
            Ec = small.tile([128, NQ, NSLOT + 1], F32, tag="Ec")
            scr = small.tile([128, NSLOT], F32, tag="scr")
            mb4 = small.tile([128, NQ], F32, tag="mb4")
            sums4 = small.tile([128, NQ], F32, tag="sums4")
            gvd4 = small.tile([128, NQ], F32, tag="gvd4")
            st4 = small.tile([128, 6, NQ], F32, tag="st4")
            mx4, r4, t4, gl4, en4, g4 = (st4[:, i, :] for i in range(6))
            cp4 = small.tile([128, NQ], F32, tag="cp4")
            nc.vector.tensor_reduce(mx4, Stok[:, :, 0:NSLOT], axis=AX_X.X, op=ALU.max)
            nc.vector.tensor_scalar_mul(mb4, mx4, -10.0)
            for q in range(NQ):
                nc.scalar.activation(
                    Etok[q],
                    Stok[:, q, 0:NSLOT],
                    func=ACTF.Exp,
                    bias=mb4[:, q : q + 1],
                    scale=10.0,
                    accum_out=sums4[:, q : q + 1],
                )
            for q in range(NQ):
                nc.vector.tensor_mul(scr, Etok[q], gv_rep)
                nc.vector.tensor_reduce(
                    gvd4[:, q : q + 1], scr, axis=AX_X.X, op=ALU.add
                )
            nc.vector.reciprocal(r4, sums4)
            nc.vector.tensor_mul(t4, gvd4, r4)
            nc.vector.tensor_add(gl4, t4, Stok[:, :, NSLOT])
            nc.scalar.activation(en4, gl4, func=ACTF.Exp, bias=-gate_b, scale=-1.0)
            nc.vector.tensor_scalar_add(g4, en4, 1.0)
            nc.vector.reciprocal(g4, g4)
            nc.vector.tensor_mul(cp4, r4, en4)
            nc.vector.tensor_scalar_mul(cp4, cp4, -1.0)
            for q in range(NQ):
                nc.vector.tensor_scalar_mul(
                    Ec[:, q, 0:NSLOT], Etok[q], cp4[:, q : q + 1]
                )
            for q in range(NQ):
                nc.vector.tensor_copy(Ec[:, q, NSLOT : NSLOT + 1], g4[:, q : q + 1])
            st["Ec"] = Ec

        def phase_C(t, st, s_next=None):
            """Ec -> slot-major; PSUM = xh + Ec'@WvN; out = g*PSUM; DMA.

            s_next=(t2, st2): interleave tile t2's S matmuls between this
            tile's I/R matmuls so PE stays busy while DVE paces the combine
            (PSUM groups are per-bank, so this is legal).
            """
            x_sb, Ec = st["x_sb"], st["Ec"]
            ET = psE.tile([NSLOT + 1, NQ, 128], F32, tag="E")
            for q in range(NQ):
                nc.tensor.transpose(ET[:, q, :], Ec[:, q, :], ident)
            E_sb = work.tile([NSLOT + 1, NQ, 128], F16, tag="E_sb")
            nc.scalar.copy(E_sb, ET)
            E_flat = E_sb.rearrange("p a b -> p (a b)")  # [65, 512]
            g_bc = psG.tile([128, TOK], F32, tag="G")
            nc.tensor.matmul(
                g_bc,
                ones16[NSLOT : NSLOT + 1, :],
                E_flat[NSLOT : NSLOT + 1, :],
                start=True,
                stop=True,
            )
            g_sb = work.tile([128, TOK], F32, tag="g_sb")
            nc.scalar.copy(g_sb, g_bc)
            if s_next is not None:
                t2, st2 = s_next
                S_ps2 = psA.tile([128, TOK], F32, tag="A")
            for a in range(4):
                o4 = opool.tile([128, 4, TOK], F16, tag="o4")
                for cc in range(4):
                    c = 4 * a + cc
                    R_ps = psR.tile([128, TOK], F32, tag="R")
                    nc.tensor.matmul(
                        R_ps,
                        ident_i,
                        x_sb[:, c, :],
                        start=True,
                        stop=False,
                        skip_group_check=True,
                    )
                    nc.tensor.matmul(
                        R_ps,
                        Wv_sb[:, c, :],
                        E_flat[0:NSLOT, :],
                        start=False,
                        stop=True,
                        skip_group_check=True,
                    )
                    if s_next is not None:
                        nc.tensor.matmul(
                            S_ps2[0 : NSLOT + 1, :],
                            GT_sb[:, c, :],
                            st2["x_sb"][:, c, :],
                            start=(c == 0),
                            stop=(c == NCH - 1),
                            skip_group_check=True,
                        )
                    nc.vector.tensor_mul(o4[:, cc, :], R_ps, g_sb)
                nc.scalar.dma_start(
                    out=outT_v[:, 4 * a : 4 * a + 4, ts(t, TOK)],
                    in_=o4,
                )
            if s_next is not None:
                t2, st2 = s_next
                if split8:
                    S2_ps2 = psA2.tile([128, TOK], F32, tag="A2")
                    S_corr_mms(st2, S2_ps2)
                else:
                    S2_ps2 = None
                S_merge(st2, S_ps2, S2_ps2)

        # software pipeline, 3 tiles deep: x-DMA 3 tiles ahead; tile t+2's
        # S matmuls are interleaved into tile t's combine phase; tile t+2's
        # stats are issued AFTER phase_C(t) so they never sit ahead of the
        # combine ops in the in-order DVE queue (that ordering starved the
        # PE of PSUM banks every tile and HAM-downclocked it to 1.2 GHz).
        states = {}
        states[0] = phase_A_dma(0)
        states[1] = phase_A_dma(1)
        states[2] = phase_A_dma(2)
        phase_A_mm(0, states[0])
        phase_A_mm(1, states[1])
        phase_B(0, states[0])
        phase_B(1, states[1])
        for t in range(NT):
            if t + 3 < NT:
                states[t + 3] = phase_A_dma(t + 3)
            phase_C(
                t,
                states[t],
                s_next=(t + 2, states[t + 2]) if t + 2 < NT else None,
            )
            if t + 2 < NT:
                phase_B(t + 2, states[t + 2])
            del states[t]

    nc.compile()
    return nc


def _fold_weights(memory, key_w, value_w, gate_w, split8=True):
    mem = memory.astype(np.float64)
    Ws = (mem @ key_w.astype(np.float64)).astype(np.float32)  # [64, 2048]
    Wv = (mem @ value_w.astype(np.float64).T).astype(np.float32)  # [64, 2048]
    gx = np.asarray(gate_w[0, :DIM], dtype=np.float32)
    gvv = (Wv.astype(np.float64) @ gate_w[0, DIM:].astype(np.float64)).astype(
        np.float32
    )
    G = np.concatenate([Ws, gx[None, :]], axis=0)  # [65, 2048]; gate row last
    WvN = np.ascontiguousarray(-Wv).astype(np.float16)  # [64, 2048]
    if not split8:
        GT = np.ascontiguousarray(G.T)  # [2048, 65] fp32
        return GT, None, WvN, gvv.reshape(1, NSLOT)
    F8NP = mybir.dt.np(F8)
    Gh = G.astype(np.float16)  # [65, 2048]
    Gh8 = Gh.astype(F8NP)
    Gl10 = ((G - Gh.astype(np.float32)) * CSCALE).astype(F8NP)
    GT = np.ascontiguousarray(Gh.T)  # [2048, 65] fp16
    GC = np.zeros((2 * DIM, MPAD), dtype=F8NP)
    GC[:DIM, : NSLOT + 1] = Gh8.T
    GC[DIM:, : NSLOT + 1] = Gl10.T
    # [4096, 80] e4m3 (cols 65..79 zero-padded for the DoubleRow step%16==0
    # LDW restriction): rows 0..2047 -> xl term, 2048..4095 -> xh term
    return GT, GC, WvN, gvv.reshape(1, NSLOT)


def kernel(
    x, memory, key_w, value_w, gate_w, gate_b,
    _trace=False, _tmpdir=None, _score_mode="split8", _cvt_pool=16,
):
    x = np.asarray(x, dtype=np.float32)
    split8 = _score_mode == "split8"
    GT, GC, WvN, gvv = _fold_weights(
        np.asarray(memory, np.float32),
        np.asarray(key_w, np.float32),
        np.asarray(value_w, np.float32),
        np.asarray(gate_w, np.float32),
        split8=split8,
    )
    nc = _build(
        float(np.asarray(gate_b).reshape(-1)[0]),
        score_mode=_score_mode,
        cvt_pool=_cvt_pool,
    )
    F8NP = mybir.dt.np(F8)
    in_maps = []
    for b in range(B):
        xT32 = np.ascontiguousarray(x[b].T)
        if split8:
            xh = xT32.astype(np.float16)
            xl8 = ((xT32 - xh.astype(np.float32)) * CSCALE).astype(F8NP)
            m = {"xT": xh, "xl": xl8, "xh8": xh.astype(F8NP),
                 "GT": GT, "GC": GC, "WvN": WvN, "gv": gvv}
        else:
            m = {"xT": xT32, "GT": GT, "WvN": WvN, "gv": gvv}
        in_maps.append(m)
    res = run_bass_kernel_spmd(
        nc, in_maps, list(range(B)), trace=_trace, tmpdir=_tmpdir
    )
    out = np.stack(
        [res.results[b]["outT"].T.astype(np.float32) for b in range(B)], axis=0
    )
    if _trace:
        return out, res
    return out

